# revision 8
# baseline (speedup 1.0000x reference)
"""Trainium2 Bass kernel for nn_CrossAttentionPoseRegression_11742440587710.

Strategy (8 NeuronCores, data-sharded over the N=524288 points):
  Launch 1 (per core, N/8 points): stream h_src/h_tgt shards; compute
    sim = <hs,ht> per point, per-point norms, cos -> feature-loss partial,
    chamfer -> rotation-loss partial. Ships sim (256KB/core) + per-partition
    loss partials back to host.
  Host glue: exact top-k(128) of sim, 128-row scoring MLP, the
    replace/cond logic, softmax scalars alpha=1/T1 and m=max(fw*alpha).
  Launch 2 (per core): e = exp(sim*alpha - m); raw weighted Kabsch moment
    partial sums (S, Sx, St, M=sum e*xs*xt^T) per partition.
  Host tail: corrections for the <=128 replaced top-k entries (old/new exp
    contributions), normalization, 3x3 SVD -> (Rb, tb).
  h_src/x_src/h_tgt/x_tgt outputs are identity pass-throughs of the inputs.
"""

import numpy as np

import concourse.bass as bass
import concourse.tile as tile
from concourse import bacc, mybir
from concourse.bass_utils import run_bass_kernel_spmd

N = 524288
K = 128
HF = 33
NCORES = 8
SH = N // NCORES          # 65536 points per core
P = 128                   # SBUF partitions
A = SH // P               # 512 points per partition
NCHUNK = 8
CA = A // NCHUNK          # 64 points (columns) per chunk

dt = mybir.dt
f32 = np.float32
Alu = mybir.AluOpType
Act = mybir.ActivationFunctionType
Ax = mybir.AxisListType


def _build_launch1():
    nc = bacc.Bacc("TRN2", target_bir_lowering=False, debug=False)
    hs_d = nc.dram_tensor("hs", [SH, HF], dt.float32, kind="ExternalInput").ap()
    ht_d = nc.dram_tensor("ht", [SH, HF], dt.float32, kind="ExternalInput").ap()
    xs_d = nc.dram_tensor("xs", [SH, 3], dt.float32, kind="ExternalInput").ap()
    xt_d = nc.dram_tensor("xt", [SH, 3], dt.float32, kind="ExternalInput").ap()
    lab_d = nc.dram_tensor("lab", [SH], dt.float32, kind="ExternalInput").ap()
    # rt[p, :] = [R00..R22, t0, t1, t2] replicated on every partition (host-side)
    rt_d = nc.dram_tensor("rt", [P, 12], dt.float32, kind="ExternalInput").ap()
    sim_d = nc.dram_tensor("sim", [P, A], dt.float32, kind="ExternalOutput").ap()
    # stats cols: 0..2 = rot partial per axis, 3 = feat partial
    st_d = nc.dram_tensor("st", [P, 4], dt.float32, kind="ExternalOutput").ap()

    hs3 = hs_d.rearrange("(p a) f -> p a f", p=P)
    ht3 = ht_d.rearrange("(p a) f -> p a f", p=P)
    xs3 = xs_d.rearrange("(p a) f -> p a f", p=P)
    xt3 = xt_d.rearrange("(p a) f -> p a f", p=P)
    lab2 = lab_d.rearrange("(p a) -> p a", p=P)

    with tile.TileContext(nc) as tc:
        with (
            tc.tile_pool(name="persist", bufs=1) as pp,
            tc.tile_pool(name="hin", bufs=3) as hp,
            tc.tile_pool(name="sq", bufs=2) as qp,
            tc.tile_pool(name="scratch", bufs=2) as sp,
        ):
            sim_sb = pp.tile([P, A], dt.float32, tag="sim")
            ns_sb = pp.tile([P, A], dt.float32, tag="ns")
            nt_sb = pp.tile([P, A], dt.float32, tag="nt")
            st_sb = pp.tile([P, 4], dt.float32, tag="st")
            xs_t = pp.tile([P, A * 3], dt.float32, tag="xs")
            xt_t = pp.tile([P, A * 3], dt.float32, tag="xt")
            lab_t = pp.tile([P, A], dt.float32, tag="lab")
            rt_t = pp.tile([P, 12], dt.float32, tag="rt")

            nc.sync.dma_start(xs_t[:], xs3[:, :, :])
            nc.sync.dma_start(xt_t[:], xt3[:, :, :])
            nc.sync.dma_start(lab_t[:], lab2[:, :])
            nc.sync.dma_start(rt_t[:], rt_d[:, :])

            xs_v = xs_t[:].rearrange("p (a f) -> p a f", f=3)
            xt_v = xt_t[:].rearrange("p (a f) -> p a f", f=3)

            # ---- h pass: sim + norms, chunked over columns ----
            for i in range(NCHUNK):
                c0, c1 = i * CA, (i + 1) * CA
                hs_t = hp.tile([P, CA * HF], dt.float32, tag="hs")
                nc.sync.dma_start(hs_t[:], hs3[:, c0:c1, :])
                ht_t = hp.tile([P, CA * HF], dt.float32, tag="ht")
                nc.sync.dma_start(ht_t[:], ht3[:, c0:c1, :])

                prod = sp.tile([P, CA * HF], dt.float32, tag="prod")
                nc.gpsimd.tensor_mul(prod[:], hs_t[:], ht_t[:])
                ss = qp.tile([P, CA * HF], dt.float32, tag="ss")
                nc.scalar.square(ss[:], hs_t[:])
                tt = qp.tile([P, CA * HF], dt.float32, tag="tt")
                nc.scalar.square(tt[:], ht_t[:])

                prod3 = prod[:].rearrange("p (a f) -> p a f", f=HF)
                ss3 = ss[:].rearrange("p (a f) -> p a f", f=HF)
                tt3 = tt[:].rearrange("p (a f) -> p a f", f=HF)

                # per-point reduce over 33 = reduce(32 even cols) + add last col
                nc.vector.reduce_sum(sim_sb[:, c0:c1], prod3[:, :, 0:32], axis=Ax.X)
                nc.vector.tensor_add(sim_sb[:, c0:c1], sim_sb[:, c0:c1], prod3[:, :, 32])
                nc.vector.reduce_sum(ns_sb[:, c0:c1], ss3[:, :, 0:32], axis=Ax.X)
                nc.vector.tensor_add(ns_sb[:, c0:c1], ns_sb[:, c0:c1], ss3[:, :, 32])
                nc.vector.reduce_sum(nt_sb[:, c0:c1], tt3[:, :, 0:32], axis=Ax.X)
                nc.vector.tensor_add(nt_sb[:, c0:c1], nt_sb[:, c0:c1], tt3[:, :, 32])

            nc.sync.dma_start(sim_d[:, :], sim_sb[:])

            # ---- tail: cos + feature loss ----
            npr = sp.tile([P, A], dt.float32, tag="w1")
            nc.vector.tensor_mul(npr[:], ns_sb[:], nt_sb[:])
            lnp = sp.tile([P, A], dt.float32, tag="w2")
            nc.scalar.activation(lnp[:], npr[:], Act.Ln)
            rsq = sp.tile([P, A], dt.float32, tag="w3")
            nc.scalar.activation(rsq[:], lnp[:], Act.Exp, scale=-0.5)
            cos = sp.tile([P, A], dt.float32, tag="w4")
            nc.vector.tensor_mul(cos[:], sim_sb[:], rsq[:])
            d = sp.tile([P, A], dt.float32, tag="w5")
            nc.vector.tensor_sub(d[:], cos[:], lab_t[:])
            d2 = sp.tile([P, A], dt.float32, tag="w6")
            nc.vector.scalar_tensor_tensor(
                out=d2[:], in0=d[:], scalar=1.0, in1=d[:],
                op0=Alu.mult, op1=Alu.mult, accum_out=st_sb[:, 3:4],
            )

            # ---- x pass: chamfer * labels (rotation loss partials) ----
            for i in range(3):
                Ri0 = rt_t[:, 3 * i + 0 : 3 * i + 1]
                Ri1 = rt_t[:, 3 * i + 1 : 3 * i + 2]
                Ri2 = rt_t[:, 3 * i + 2 : 3 * i + 3]
                tgi = rt_t[:, 9 + i : 10 + i]
                b = sp.tile([P, A], dt.float32, tag="xb")
                # b = tg_i - xt_i
                nc.vector.tensor_scalar(
                    out=b[:], in0=xt_v[:, :, i], scalar1=-1.0, scalar2=tgi,
                    op0=Alu.mult, op1=Alu.add,
                )
                u = sp.tile([P, A], dt.float32, tag="xu")
                nc.vector.scalar_tensor_tensor(
                    out=u[:], in0=xs_v[:, :, 0], scalar=Ri0, in1=b[:],
                    op0=Alu.mult, op1=Alu.add,
                )
                nc.vector.scalar_tensor_tensor(
                    out=u[:], in0=xs_v[:, :, 1], scalar=Ri1, in1=u[:],
                    op0=Alu.mult, op1=Alu.add,
                )
                nc.vector.scalar_tensor_tensor(
                    out=u[:], in0=xs_v[:, :, 2], scalar=Ri2, in1=u[:],
                    op0=Alu.mult, op1=Alu.add,
                )
                ul = sp.tile([P, A], dt.float32, tag="xul")
                nc.gpsimd.tensor_mul(ul[:], u[:], lab_t[:])
                u2 = sp.tile([P, A], dt.float32, tag="xu2")
                nc.vector.scalar_tensor_tensor(
                    out=u2[:], in0=ul[:], scalar=1.0, in1=u[:],
                    op0=Alu.mult, op1=Alu.mult, accum_out=st_sb[:, i : i + 1],
                )

            nc.sync.dma_start(st_d[:, :], st_sb[:])

    nc.compile()
    return nc


def _build_launch2():
    nc = bacc.Bacc("TRN2", target_bir_lowering=False, debug=False)
    xs_d = nc.dram_tensor("xs", [SH, 3], dt.float32, kind="ExternalInput").ap()
    xt_d = nc.dram_tensor("xt", [SH, 3], dt.float32, kind="ExternalInput").ap()
    sim_d = nc.dram_tensor("sim", [P, A], dt.float32, kind="ExternalInput").ap()
    # sc[p, :] = [alpha, -m] replicated (host-side)
    sc_d = nc.dram_tensor("sc", [P, 2], dt.float32, kind="ExternalInput").ap()
    # mom cols: 0=S, 1..3=Sx, 4..6=St, 7..15=M (row major)
    mom_d = nc.dram_tensor("mom", [P, 16], dt.float32, kind="ExternalOutput").ap()

    xs3 = xs_d.rearrange("(p a) f -> p a f", p=P)
    xt3 = xt_d.rearrange("(p a) f -> p a f", p=P)

    with tile.TileContext(nc) as tc:
        with (
            tc.tile_pool(name="persist", bufs=1) as pp,
            tc.tile_pool(name="scratch", bufs=2) as sp,
        ):
            xs_t = pp.tile([P, A * 3], dt.float32, tag="xs")
            xt_t = pp.tile([P, A * 3], dt.float32, tag="xt")
            sim_t = pp.tile([P, A], dt.float32, tag="sim")
            sc_t = pp.tile([P, 2], dt.float32, tag="sc")
            mom_t = pp.tile([P, 16], dt.float32, tag="mom")
            nc.sync.dma_start(xs_t[:], xs3[:, :, :])
            nc.sync.dma_start(xt_t[:], xt3[:, :, :])
            nc.sync.dma_start(sim_t[:], sim_d[:, :])
            nc.sync.dma_start(sc_t[:], sc_d[:, :])

            xs_v = xs_t[:].rearrange("p (a f) -> p a f", f=3)
            xt_v = xt_t[:].rearrange("p (a f) -> p a f", f=3)

            e = pp.tile([P, A], dt.float32, tag="e")
            nc.scalar.activation(
                e[:], sim_t[:], Act.Exp, bias=sc_t[:, 1:2], scale=sc_t[:, 0:1]
            )
            # S = sum e
            nc.vector.reduce_sum(mom_t[:, 0:1], e[:], axis=Ax.X)
            exs = []
            for i in range(3):
                q = pp.tile([P, A], dt.float32, tag=f"exs{i}")
                nc.vector.tensor_mul(q[:], e[:], xs_v[:, :, i])
                exs.append(q)
                # Sx_i = sum e*xs_i
                nc.vector.reduce_sum(mom_t[:, 1 + i : 2 + i], q[:], axis=Ax.X)
            for j in range(3):
                # St_j = sum e*xt_j  (fused mul+reduce)
                scr = sp.tile([P, A], dt.float32, tag="scr")
                nc.vector.scalar_tensor_tensor(
                    out=scr[:], in0=e[:], scalar=1.0, in1=xt_v[:, :, j],
                    op0=Alu.mult, op1=Alu.mult,
                    accum_out=mom_t[:, 4 + j : 5 + j],
                )
            for i in range(3):
                for j in range(3):
                    scr = sp.tile([P, A], dt.float32, tag="scr2")
                    nc.vector.scalar_tensor_tensor(
                        out=scr[:], in0=exs[i][:], scalar=1.0, in1=xt_v[:, :, j],
                        op0=Alu.mult, op1=Alu.mult,
                        accum_out=mom_t[:, 7 + 3 * i + j : 8 + 3 * i + j],
                    )
            nc.sync.dma_start(mom_d[:, :], mom_t[:])

    nc.compile()
    return nc


_CACHE = {}


def _get_programs():
    if "nc1" not in _CACHE:
        _CACHE["nc1"] = _build_launch1()
        _CACHE["nc2"] = _build_launch2()
    return _CACHE["nc1"], _CACHE["nc2"]


def kernel(**inputs):
    nc1, nc2 = _get_programs()

    h_src = np.asarray(inputs["h_src"])
    x_src = np.asarray(inputs["x_src"])
    h_tgt = np.asarray(inputs["h_tgt"])
    x_tgt = np.asarray(inputs["x_tgt"])
    labels = np.asarray(inputs["labels"])
    gt_pose = np.asarray(inputs["gt_pose"])

    hs = np.ascontiguousarray(h_src[0], f32)
    ht = np.ascontiguousarray(h_tgt[0], f32)
    xs = np.ascontiguousarray(x_src[0], f32)
    xt = np.ascontiguousarray(x_tgt[0], f32)
    lab = np.ascontiguousarray(labels[0], f32)
    gp = np.asarray(gt_pose[0], f32)
    Rg, tg = gp[:3, :3], gp[:3, -1]
    rt = np.broadcast_to(
        np.concatenate([Rg.reshape(9), tg]).astype(f32)[None, :], (P, 12)
    ).copy()

    in_maps1 = []
    for c in range(NCORES):
        sl = slice(c * SH, (c + 1) * SH)
        in_maps1.append({
            "hs": hs[sl], "ht": ht[sl], "xs": xs[sl], "xt": xt[sl],
            "lab": lab[sl], "rt": rt,
        })
    res1 = run_bass_kernel_spmd(nc1, in_maps1, list(range(NCORES))).results

    sim_shards = [res1[c]["sim"] for c in range(NCORES)]        # [P, A] each
    sim = np.concatenate([s.reshape(SH) for s in sim_shards])    # (N,)
    stats = np.stack([res1[c]["st"] for c in range(NCORES)])     # (NC, P, 4)
    rot_sum = stats[:, :, 0:3].astype(np.float64).sum()
    feat_sum = stats[:, :, 3].astype(np.float64).sum()
    total_loss = f32(rot_sum / N + feat_sum / N)

    # ---- host glue: topk + MLP + replace logic + softmax scalars ----
    top_idx = np.argpartition(-sim, K)[:K]
    top_idx = top_idx[np.argsort(-sim[top_idx], kind="stable")]
    tkv = sim[top_idx]
    W1 = np.asarray(inputs["W1"], f32); b1 = np.asarray(inputs["b1"], f32)
    W2 = np.asarray(inputs["W2"], f32); b2 = np.asarray(inputs["b2"], f32)
    W3 = np.asarray(inputs["W3"], f32); b3 = np.asarray(inputs["b3"], f32)
    z = np.concatenate([hs[top_idx], ht[top_idx]], -1)
    z = np.maximum(z @ W1.T + b1, 0).astype(f32)
    z = np.maximum(z @ W2.T + b2, 0).astype(f32)
    pred = ((z @ W3.T + b3)[:, 0] / K).astype(f32)
    p0 = pred[0]
    cond = (p0 > 0.5) & ((np.abs(p0 - f32(1.0)) < tkv) | (p0 < tkv))
    replaced = np.where(cond, p0, tkv).astype(f32)

    fw_sum = f32(sim.sum(dtype=f32))
    delta = (replaced.astype(np.float64) - tkv.astype(np.float64)).sum()
    T1 = f32(fw_sum + delta + 1e-6)
    alpha = f32(1.0) / T1
    # m = max over fw*alpha where fw = sim with top entries replaced
    fa = sim * alpha
    fa[top_idx] = replaced * alpha
    m = f32(fa.max())

    sc = np.broadcast_to(np.array([alpha, -m], f32)[None, :], (P, 2)).copy()

    in_maps2 = []
    for c in range(NCORES):
        sl = slice(c * SH, (c + 1) * SH)
        in_maps2.append({
            "xs": xs[sl], "xt": xt[sl], "sim": sim_shards[c], "sc": sc,
        })
    res2 = run_bass_kernel_spmd(nc2, in_maps2, list(range(NCORES))).results
    mom = np.stack([res2[c]["mom"] for c in range(NCORES)]).astype(np.float64)
    mom = mom.sum(axis=(0, 1))                                   # (16,)
    S1 = mom[0]
    Sx = mom[1:4].copy()
    St = mom[4:7].copy()
    M = mom[7:16].reshape(3, 3).copy()

    # ---- corrections for replaced entries ----
    e_old = np.exp(tkv * alpha - m, dtype=f32).astype(np.float64)
    e_new = np.exp(replaced * alpha - m, dtype=f32).astype(np.float64)
    de = e_new - e_old
    if np.any(de != 0.0):
        xs_k = xs[top_idx].astype(np.float64)
        xt_k = xt[top_idx].astype(np.float64)
        S1 += de.sum()
        Sx += (de[:, None] * xs_k).sum(0)
        St += (de[:, None] * xt_k).sum(0)
        M += np.einsum("n,ni,nj->ij", de, xs_k, xt_k)

    # ---- host tail: normalization + weighted Kabsch + SVD ----
    # w = e/Z; s2 = sum(w) (f32); w2 = w/(s2 + 1e-6)
    Z = S1
    # emulate f32 sum of w = e/Z: s2 ≈ 1 with f32 rounding
    e_full = np.exp(sim * alpha - m, dtype=f32)
    e_full[top_idx] = e_new.astype(f32)
    s2 = float((e_full / f32(Z)).sum(dtype=f32))
    denom = Z * (s2 + 1e-6)
    cs = Sx / denom
    ct = St / denom
    S2 = s2 / (s2 + 1e-6)
    H = M / denom - np.outer(cs, ct) * (2.0 - S2)
    H = H + 1e-6 * np.eye(3)
    U, _, Vt = np.linalg.svd(H)
    R0 = Vt.T @ U.T
    sign = -1.0 if np.linalg.det(R0) < 0 else 1.0
    Vt2 = Vt.copy()
    Vt2[-1, :] *= sign
    Rb = (Vt2.T @ U.T).astype(f32)
    tb = (ct - Rb.astype(np.float64) @ cs).astype(f32)

    return (
        Rb[None],
        tb[None],
        np.float32(total_loss),
        h_src,
        x_src,
        h_tgt,
        x_tgt,
    )
